# revision 1
# baseline (speedup 1.0000x reference)
"""PointPillars Trainium2 kernel: pillar MLP + masked-max + BEV scatter + 2 convs + head.

Strategy: batch=1, so shard the 512x512 BEV canvas into 8 horizontal bands
(64 input rows -> 32 output rows each) with halo rows so no cross-core
communication is needed.  Host-side prep folds the point mask into an 8th
input feature (emb = relu(max_p(X8 @ W8))), dedups duplicate cells
(last-wins, matching XLA scatter-set), and lays pillars out in per-canvas-row
slots so the device-side scatter becomes one-hot matmuls on the TensorEngine.
"""

import os
import sys
from contextlib import ExitStack

sys.path.insert(0, "/opt/trn_rl_repo")

import numpy as np
import ml_dtypes

import concourse.bass as bass
import concourse.tile as tile
from concourse import bacc, mybir
from concourse.masks import make_identity
from concourse.bass_utils import run_bass_kernel_spmd

BF16 = mybir.dt.bfloat16
F32 = mybir.dt.float32
NPBF16 = ml_dtypes.bfloat16

NCORES = 8
H = W = 512
P = 32            # points per pillar
CE = 64           # embedding channels
ROWS = 72         # scatter rows per core (64 owned + halos)
SLOTS = 64        # pillar slots per canvas row (measured max is 54)
NPC = ROWS * SLOTS          # 4608 pillar slots per core
GROUPS = 16                 # x8t partition groups (8 partitions each)
GCOLS = NPC * P // GROUPS   # 9216 moving columns per group
CHUNK = 512
NCH = GCOLS // CHUNK        # 18 psum chunks per group
TILES = NPC // 128          # 36 transpose tiles
C1R = 34                    # conv1 output rows (32 owned + 2 halo)
OUTR = 32                   # owned output rows per core
CANW = 514                  # canvas width: 512 data + 2 right zero-pad


# ----------------------------------------------------------------------------
# device program
# ----------------------------------------------------------------------------

def _build_program():
    stages = set(os.environ.get(
        "KSTAGES", "pfn,tr,scat,conv1,conv2,head").split(","))
    nc = bacc.Bacc(None, target_bir_lowering=False, debug=False)

    x8t = nc.dram_tensor("x8t", [8, NPC * P], BF16, kind="ExternalInput")
    colv = nc.dram_tensor("colv", [128, ROWS], F32, kind="ExternalInput")
    w8 = nc.dram_tensor("w8", [8, CE], BF16, kind="ExternalInput")
    wc1 = nc.dram_tensor("wc1", [CE, 9 * 128], BF16, kind="ExternalInput")
    b1v = nc.dram_tensor("b1v", [128, 1], F32, kind="ExternalInput")
    wc2 = nc.dram_tensor("wc2", [128, 9 * 128], BF16, kind="ExternalInput")
    b2v = nc.dram_tensor("b2v", [128, 1], F32, kind="ExternalInput")
    whd = nc.dram_tensor("whd", [128, 34], BF16, kind="ExternalInput")
    bhd = nc.dram_tensor("bhd", [34, 1], F32, kind="ExternalInput")
    rmask = nc.dram_tensor("rmask", [128, 2], F32, kind="ExternalInput")
    out = nc.dram_tensor("out", [34, OUTR, 256], F32, kind="ExternalOutput")

    with tile.TileContext(nc) as tc, ExitStack() as ctx:
        const = ctx.enter_context(tc.tile_pool(name="const", bufs=1))
        big = ctx.enter_context(tc.tile_pool(name="big", bufs=1))
        sc = ctx.enter_context(tc.tile_pool(name="scratch", bufs=3))
        stg = ctx.enter_context(tc.tile_pool(name="staging", bufs=2))
        ps = ctx.enter_context(tc.tile_pool(name="psum", bufs=2, space="PSUM"))

        # ---- constants in ----
        w8_sb = const.tile([8, CE], BF16)
        nc.gpsimd.dma_start(w8_sb[:], w8[:])
        wc1_sb = const.tile([CE, 9 * 128], BF16)
        nc.gpsimd.dma_start(wc1_sb[:], wc1[:])
        b1_sb = const.tile([128, 1], F32)
        nc.gpsimd.dma_start(b1_sb[:], b1v[:])
        wc2_sb = const.tile([128, 9 * 128], BF16)
        nc.gpsimd.dma_start(wc2_sb[:], wc2[:])
        b2_sb = const.tile([128, 1], F32)
        nc.gpsimd.dma_start(b2_sb[:], b2v[:])
        whd_sb = const.tile([128, 34], BF16)
        nc.gpsimd.dma_start(whd_sb[:], whd[:])
        bhd_sb = const.tile([34, 1], F32)
        nc.gpsimd.dma_start(bhd_sb[:], bhd[:])
        colv_sb = const.tile([128, ROWS], F32)
        nc.gpsimd.dma_start(colv_sb[:], colv[:])
        rmask_sb = const.tile([128, 2], F32)
        nc.gpsimd.dma_start(rmask_sb[:], rmask[:])

        ident = const.tile([CE, CE], BF16)
        make_identity(nc, ident[:])
        iota_i = const.tile([128, 512], mybir.dt.int32)
        nc.gpsimd.iota(iota_i[:], pattern=[[1, 512]], base=0, channel_multiplier=0)
        iota_f = const.tile([128, 512], F32)
        nc.vector.tensor_copy(iota_f[:], iota_i[:])

        # ---- big buffers ----
        emb = big.tile([CE, NPC], BF16)
        embT = big.tile([128, TILES * CE], BF16)
        canvas = big.tile([CE, ROWS, CANW], BF16)
        out2 = big.tile([128, OUTR, 256], BF16)

        # ---- PFN: emb[c, slot] = max_p (X8 @ W8), relu at the end ----
        for g in range(GROUPS if "pfn" in stages else 0):
            stage = stg.tile([8, GCOLS], BF16, tag="stage")
            nc.gpsimd.dma_start(stage[:], x8t[:, g * GCOLS : (g + 1) * GCOLS])
            for ch in range(NCH):
                pt = ps.tile([CE, 16, P], F32, tag="pfn")
                nc.tensor.matmul(
                    pt[:],
                    lhsT=w8_sb[:],
                    rhs=stage[:, ch * CHUNK : (ch + 1) * CHUNK],
                    start=True,
                    stop=True,
                )
                slot0 = g * (GCOLS // P) + ch * 16
                nc.vector.tensor_reduce(
                    emb[:, slot0 : slot0 + 16].unsqueeze(2),
                    pt[:],
                    axis=mybir.AxisListType.X,
                    op=mybir.AluOpType.max,
                )
        if "pfn" in stages:
            nc.scalar.activation(emb[:], emb[:], mybir.ActivationFunctionType.Relu)

        # ---- transpose emb into pillar-major layout for scatter matmuls ----
        for t in range(TILES if "tr" in stages else 0):
            ptr = ps.tile([128, CE], BF16, tag="tr")
            nc.tensor.transpose(ptr[:], emb[:, 128 * t : 128 * (t + 1)], ident[:])
            nc.scalar.copy(embT[:, CE * t : CE * (t + 1)], ptr[:])

        # ---- scatter: canvas[c, r, x] = sum_slots emb * onehot(col == x) ----
        for r in range(ROWS if "scat" in stages else 0):
            oh = sc.tile([128, 512], BF16, tag="oh")
            nc.vector.tensor_scalar(
                oh[:], iota_f[:], colv_sb[:, r : r + 1], None,
                op0=mybir.AluOpType.is_equal,
            )
            pss = ps.tile([CE, 512], F32, tag="pfn")
            t = r // 2
            nc.tensor.matmul(
                pss[:], lhsT=embT[:, CE * t : CE * (t + 1)], rhs=oh[:],
                start=True, stop=True,
            )
            nc.scalar.copy(canvas[:, r, 0:512], pss[:])
        nc.vector.memset(canvas[:, :, 512:514], 0.0)

        # ---- conv1: 64->128, 3x3, stride 2 (XLA SAME: pad_lo=0, pad_hi=1) ----
        out1 = big.tile([128, C1R, 258], BF16, tag="x8t_out1")
        nc.vector.memset(out1[:, :, 0:1], 0.0)
        nc.vector.memset(out1[:, :, 257:258], 0.0)
        for chk in range(C1R // 2 if "conv1" in stages else 0):
            o = 2 * chk
            pc = ps.tile([128, 2, 256], F32, tag="mm")
            for k in range(9):
                dy, dx = k // 3, k % 3
                rhs = canvas[:, 2 * o + dy : 2 * o + dy + 3 : 2, dx : dx + 512 : 2]
                nc.tensor.matmul(
                    pc[:], lhsT=wc1_sb[:, 128 * k : 128 * (k + 1)], rhs=rhs,
                    start=(k == 0), stop=(k == 8),
                )
            nc.scalar.activation(
                out1[:, o : o + 2, 1:257], pc[:],
                mybir.ActivationFunctionType.Relu, bias=b1_sb[:],
            )
        # zero the conv1 halo rows that are conv2 SAME-padding at global edges
        if "conv1" in stages:
            nc.vector.tensor_scalar_mul(out1[:, 0:1, :], out1[:, 0:1, :], rmask_sb[:, 0:1])
            nc.vector.tensor_scalar_mul(out1[:, 33:34, :], out1[:, 33:34, :], rmask_sb[:, 1:2])

        # ---- conv2: 128->128, 3x3, stride 1 (symmetric SAME) ----
        for chk in range(OUTR // 2 if "conv2" in stages else 0):
            o = 2 * chk
            pc2 = ps.tile([128, 2, 256], F32, tag="mm")
            for k in range(9):
                dy, dx = k // 3, k % 3
                rhs = out1[:, o + dy : o + dy + 2, dx : dx + 256]
                nc.tensor.matmul(
                    pc2[:], lhsT=wc2_sb[:, 128 * k : 128 * (k + 1)], rhs=rhs,
                    start=(k == 0), stop=(k == 8),
                )
            nc.scalar.activation(
                out2[:, o : o + 2, :], pc2[:],
                mybir.ActivationFunctionType.Relu, bias=b2_sb[:],
            )

        # ---- head: 1x1 convs (cls 20ch + box 14ch stacked) ----
        for chk in range(OUTR // 2 if "head" in stages else 0):
            o = 2 * chk
            ph = ps.tile([34, 2, 256], F32, tag="mm")
            nc.tensor.matmul(
                ph[:], lhsT=whd_sb[:], rhs=out2[:, o : o + 2, :],
                start=True, stop=True,
            )
            hstage = sc.tile([34, 2, 256], F32, tag="hstage")
            nc.vector.tensor_scalar(
                hstage[:], ph[:], bhd_sb[:], None,
                op0=mybir.AluOpType.add,
            )
            nc.gpsimd.dma_start(out[:, o : o + 2, :], hstage[:])

    nc.compile()
    return nc


# ----------------------------------------------------------------------------
# host-side prep
# ----------------------------------------------------------------------------

def _prep_inputs(pillar_features, mask, coords, w_pfn, b_pfn,
                 w1, b1, w2, b2, w_cls, b_cls, w_box, b_box):
    pf = np.asarray(pillar_features, np.float32)
    mk = np.asarray(mask, bool)
    xy = np.asarray(coords)
    x, y = xy[:, 0].astype(np.int64), xy[:, 1].astype(np.int64)
    n = pf.shape[0]

    valid = (x >= 0) & (x < W) & (y >= 0) & (y < H)
    lin = y * W + x
    # last-wins dedup among valid pillars (matches XLA scatter .set order)
    vidx = np.nonzero(valid)[0]
    order = vidx[np.argsort(lin[vidx], kind="stable")]
    ls = lin[order]
    is_last = np.ones(len(order), bool)
    if len(order) > 1:
        is_last[:-1] = ls[1:] != ls[:-1]
    keep = order[is_last]
    # empty pillars write all-zero -> same as not writing (canvas starts 0)
    keep = keep[mk[keep].any(1)]

    mkf = mk[keep].astype(np.float32)                      # (k, 32)
    x8 = np.concatenate([pf[keep] * mkf[:, :, None], mkf[:, :, None]], axis=2)
    kx, ky = x[keep], y[keep]

    in_maps = []
    for i in range(NCORES):
        y0 = 64 * i - 2
        sel = (ky >= y0) & (ky < y0 + ROWS)
        r = (ky[sel] - y0).astype(np.int64)
        cx = kx[sel]
        xf = x8[sel]                                       # (m, 32, 8)
        # slot = rank within its row
        o2 = np.argsort(r, kind="stable")
        r_s, cx_s, xf_s = r[o2], cx[o2], xf[o2]
        row_start = np.searchsorted(r_s, np.arange(ROWS))
        slot = np.arange(len(r_s)) - row_start[r_s]
        if len(slot) and slot.max() >= SLOTS:
            raise RuntimeError(f"slot overflow: {slot.max()} >= {SLOTS}")
        p_idx = r_s * SLOTS + slot

        x8t_full = np.zeros((8, NPC * P), np.float32)
        cols = (p_idx[:, None] * P + np.arange(P)[None, :]).ravel()
        x8t_full[:, cols] = xf_s.transpose(2, 0, 1).reshape(8, -1)
        x8t_packed = x8t_full.astype(NPBF16)

        colv = np.full((128, ROWS), -1.0, np.float32)
        colv[64 * (r_s % 2) + slot, r_s] = cx_s.astype(np.float32)

        rmask = np.ones((128, 2), np.float32)
        if i == 0:
            rmask[:, 0] = 0.0
        if i == NCORES - 1:
            rmask[:, 1] = 0.0

        in_maps.append({
            "x8t": x8t_packed,
            "colv": colv,
            "rmask": rmask,
        })

    # shared weights
    w8 = np.concatenate([np.asarray(w_pfn, np.float32),
                         np.asarray(b_pfn, np.float32)[None, :]], 0).astype(NPBF16)
    wc1 = np.ascontiguousarray(
        np.asarray(w1, np.float32).transpose(2, 3, 1, 0).reshape(9, CE, 128)
        .transpose(1, 0, 2).reshape(CE, 9 * 128)
    ).astype(NPBF16)
    wc2 = np.ascontiguousarray(
        np.asarray(w2, np.float32).transpose(2, 3, 1, 0).reshape(9, 128, 128)
        .transpose(1, 0, 2).reshape(128, 9 * 128)
    ).astype(NPBF16)
    whd = np.ascontiguousarray(np.concatenate(
        [np.asarray(w_cls, np.float32)[:, :, 0, 0],
         np.asarray(w_box, np.float32)[:, :, 0, 0]], 0).T).astype(NPBF16)
    bhd = np.concatenate([np.asarray(b_cls, np.float32),
                          np.asarray(b_box, np.float32)])[:, None].astype(np.float32)
    b1c = np.asarray(b1, np.float32)[:, None]
    b2c = np.asarray(b2, np.float32)[:, None]

    for m in in_maps:
        m.update({
            "w8": w8, "wc1": wc1, "b1v": b1c, "wc2": wc2, "b2v": b2c,
            "whd": whd, "bhd": bhd,
        })
    return in_maps


_CACHE = {}


def kernel(pillar_features, mask, coords, H=None, W=None,
           w_pfn=None, b_pfn=None, w1=None, b1=None, w2=None, b2=None,
           w_cls=None, b_cls=None, w_box=None, b_box=None):
    in_maps = _prep_inputs(pillar_features, mask, coords, w_pfn, b_pfn,
                           w1, b1, w2, b2, w_cls, b_cls, w_box, b_box)
    if "nc" not in _CACHE:
        _CACHE["nc"] = _build_program()
    nc = _CACHE["nc"]

    trace = os.environ.get("KERNEL_TRACE", "0") == "1"
    res = run_bass_kernel_spmd(nc, in_maps, core_ids=list(range(NCORES)),
                               trace=trace)
    if trace and res.exec_time_ns is not None:
        print(f"HW exec time: {res.exec_time_ns} ns")
        _CACHE["exec_time_ns"] = res.exec_time_ns

    full = np.zeros((34, 256, 256), np.float32)
    for i in range(NCORES):
        full[:, 32 * i : 32 * i + 32, :] = res.results[i]["out"]
    return full[None]

